# revision 11
# baseline (speedup 1.0000x reference)
"""ViT self-attention (B=32, S=577, D=1024, H=16, Dh=64) on 8 TRN2 NeuronCores.

Sharding: data-parallel over batch — each core gets 4 batch elements, no
collectives. All matmuls run in bf16 (1 PE cycle/row vs 4 for fp32); inputs
are cast to bf16 on the host so DMA traffic halves and no on-device convert
is needed. Weights are loaded to SBUF once per core (not per batch).

Per core, per batch:
  phase 0: PE-transpose X [577,1024] -> X^T stored as one [128, 8*577] bf16
           tile (one DVE evac per 128-token tile)
  phase 1: Q^T = Wq^T X^T, K^T = Wk^T X^T (bias folded into DVE evac),
           V natural = X Wv with a ones column per head ([V_h | 1] -> softmax
           denominator falls out of the ctx matmul)
  phase 2: per head pair (row-packed K=64 matmuls at tile_position (0,0)/
           (64,0)): S^T tile = matmul(lhsT=K^T, rhs=Q^T); P^T = exp(S^T/8) on
           ACT (bf16); ctx natural = matmul(lhsT=P^T, rhs=[V_h|1]) in PSUM
           with denominator in col 64; DVE: batched recip + fused
           (ctx*recip + bv) evac; DMA out per 128-token tile.

The batch loop is software-pipelined: batch b's attention (ACT-heavy exp)
is interleaved with batch b+1's QKV projections (PE-heavy) so the scalar
engine's exp time hides under tensor-engine work.
"""

import numpy as np
import ml_dtypes

import concourse.bass as bass
import concourse.mybir as mybir
import concourse.tile as tile
from concourse.bass import ds, ts
from concourse.bass_utils import run_bass_kernel_spmd
from concourse.masks import make_identity

F32 = mybir.dt.float32
BF16 = mybir.dt.bfloat16

# ---------------------------------------------------------------------------
# Wait-legalization patch: this walrus build accepts at most ONE ge-mode sync
# wait per instruction (eq-mode counts as two). Tile's sem assignment attaches
# multi-waits directly to instructions, so hoist extras onto standalone
# EventSemaphore carriers (same engine queue, immediately preceding — identical
# semantics, queue is in-order).
# ---------------------------------------------------------------------------
_ctr = [0]


def _split_waits(insts):
    out = []
    for inst in insts:
        si = inst.sync_info
        if si is not None and si.on_wait:
            waits = list(si.on_wait)
            if len(waits) == 1 and waits[0].wait_mode != "sem-eq-imm":
                move = []
            else:
                move = waits
            for w in move:
                _ctr[0] += 1
                ev = mybir.InstEventSemaphore(
                    name=f"wsplit_{_ctr[0]}", opcode="EventSemaphore",
                    engine=inst.engine, debug=inst.debug, ins=[], outs=[],
                    sync_info=mybir.SyncInfo(on_wait=[w], on_update=[]),
                )
                out.append(ev)
            if move:
                inst.sync_info = mybir.SyncInfo(on_wait=[], on_update=list(si.on_update))
        out.append(inst)
    return out


def _install_waitfix():
    if getattr(tile.TileContext, "_waitfix_installed", False):
        return
    from concourse.vector_clock import ScopedClock

    orig_lower = tile.TileContext._lower_ordered_insts

    def patched_lower(self, ordered):
        for name in list(ordered.keys()):
            ordered[name] = _split_waits(ordered[name])
        return orig_lower(self, ordered)

    def patched_dab(self, tick_clock, wait_clock):
        nc = self.nc
        probe = nc.sync.nop(nofuse=True)
        wait_clock.add_sem_waits(probe.ins, ScopedClock({None: tick_clock.global_clock}))
        si = probe.ins.sync_info
        waits = list(si.on_wait) if si is not None else []
        probe.ins.sync_info = mybir.SyncInfo(
            on_wait=[], on_update=list(si.on_update) if si else []
        )
        for w in waits:
            _ctr[0] += 1
            ev = mybir.InstEventSemaphore(
                name=f"wsplit_dab_{_ctr[0]}", opcode="EventSemaphore",
                engine=mybir.EngineType.SP, debug=probe.ins.debug, ins=[], outs=[],
                sync_info=mybir.SyncInfo(on_wait=[w], on_update=[]),
            )
            nc.sync.add_instruction(ev)
        nc.sync.drain()
        nc.all_engine_barrier()
        assert self.sems is not None
        popped = nc._tile_sem_poison_stack.pop()
        assert popped is self._sem_poison
        nc.clear_and_free_semaphores(list(self.sems.allocated().values()))
        nc.all_engine_barrier()

    tile.TileContext._lower_ordered_insts = patched_lower
    tile.TileContext._drain_and_barrier = patched_dab
    tile.TileContext._waitfix_installed = True


_install_waitfix()

N_CORES = 8
B, S, D = 32, 577, 1024
H, Dh = 16, 64
BPC = B // N_CORES  # batches per core
S_TILES = [(t * 128, min(128, S - t * 128)) for t in range((S + 127) // 128)]  # 5 tiles
ND = D // 128  # 8 din/dout tiles
HPAIRS = H // 2
NT = len(S_TILES)

AF = mybir.ActivationFunctionType
OP = mybir.AluOpType


def build_nc():
    nc = bass.Bass()
    hidden = nc.declare_dram_parameter("hidden", [BPC, S, D], BF16, isOutput=False)
    wq = nc.declare_dram_parameter("Wq", [D, D], BF16, isOutput=False)
    bq = nc.declare_dram_parameter("bq", [D], F32, isOutput=False)
    wk = nc.declare_dram_parameter("Wk", [D, D], BF16, isOutput=False)
    bk = nc.declare_dram_parameter("bk", [D], F32, isOutput=False)
    wv = nc.declare_dram_parameter("Wv", [D, D], BF16, isOutput=False)
    bv = nc.declare_dram_parameter("bv", [D], F32, isOutput=False)
    out = nc.declare_dram_parameter("out", [BPC, S, D], F32, isOutput=True)

    with tile.TileContext(nc) as tc:
        with (
            tc.tile_pool(name="singles", bufs=1) as singles,
            tc.tile_pool(name="xnat", bufs=3) as xnat_pool,
            tc.tile_pool(name="xt", bufs=1) as xt_pool,
            tc.tile_pool(name="qt", bufs=2) as qt_pool,
            tc.tile_pool(name="kt", bufs=2) as kt_pool,
            tc.tile_pool(name="v", bufs=2) as v_pool,
            tc.tile_pool(name="pT", bufs=22) as pT_pool,
            tc.tile_pool(name="ctxT", bufs=2) as ctxT_pool,
            tc.tile_pool(name="ostage", bufs=1) as o_pool,
            tc.tile_pool(name="rc", bufs=8) as rc_pool,
            tc.tile_pool(name="psbig", bufs=3, space="PSUM") as ps_big,
            tc.tile_pool(name="psctx", bufs=2, space="PSUM") as ps_ctx,
        ):
            # --- constants ---
            identity = singles.tile([128, 128], BF16)
            make_identity(nc, identity)
            # per-dout-tile bias columns: bqt[:, m] = bq[128m : 128(m+1)]
            bqt = singles.tile([128, ND], F32)
            bkt = singles.tile([128, ND], F32)
            nc.gpsimd.dma_start(out=bqt, in_=bq[:].rearrange("(m p) -> p m", p=128))
            nc.gpsimd.dma_start(out=bkt, in_=bk[:].rearrange("(m p) -> p m", p=128))
            # bv broadcast to all 128 partitions
            bvb = singles.tile([128, D], F32)
            bv_ap = bv[:]
            nc.gpsimd.dma_start(
                out=bvb,
                in_=bass.AP(tensor=bv_ap.tensor, offset=bv_ap.offset, ap=[[0, 128]] + bv_ap.ap),
            )
            # weights: loaded once, reused for all batches
            wqb = [singles.tile([128, D], BF16, name=f"wq{k}") for k in range(ND)]
            wkb = [singles.tile([128, D], BF16, name=f"wk{k}") for k in range(ND)]
            wvb = [singles.tile([128, D], BF16, name=f"wv{k}") for k in range(ND)]
            for wdram, wtiles in ((wq, wqb), (wk, wkb), (wv, wvb)):
                for k in range(ND):
                    nc.gpsimd.dma_start(out=wtiles[k], in_=wdram[ts(k, 128), :])

            def make_tiles():
                return {
                    "xt": xt_pool.tile([128, ND * S], BF16, tag="xt", name="xt"),
                    "qt": [qt_pool.tile([128, S], BF16, tag=f"qt{m}", name=f"qtile{m}") for m in range(ND)],
                    "kt": [kt_pool.tile([128, S], BF16, tag=f"kt{m}", name=f"ktile{m}") for m in range(ND)],
                    "vt": [v_pool.tile([128, H * 65], BF16, tag=f"v{t}", name=f"vtile{t}") for t in range(NT)],
                }

            def phase01(b, tl):
                """X load + transpose, then QKV projections. Yields 9 times."""
                xt, qt, kt, vt = tl["xt"], tl["qt"], tl["kt"], tl["vt"]
                x3 = xt.rearrange("p (j c) -> p j c", c=S)
                for t, (t0, st) in enumerate(S_TILES):
                    xn = xnat_pool.tile([128, D], BF16, tag="xn")
                    nc.sync.dma_start(out=xn[:st], in_=hidden[b, t0 : t0 + st, :])
                    pst = ps_big.tile([128, 1024], BF16, tag="big", name="psbig")
                    for j in range(ND):
                        nc.tensor.transpose(
                            pst[:, j * 128 : j * 128 + st], xn[:st, ts(j, 128)], identity[:st, :st]
                        )
                    nc.vector.tensor_copy(
                        out=x3[:, :, t0 : t0 + st],
                        in_=pst.rearrange("p (j c) -> p j c", c=128)[:, :, 0:st],
                    )
                yield
                for m in range(ND):
                    for wtiles, dst, bias_t in ((wqb, qt, bqt), (wkb, kt, bkt)):
                        ps = ps_big.tile([128, 1024], F32, tag="big", name="psbig")
                        for k in range(ND):
                            nc.tensor.matmul(
                                ps[:, 0:512], wtiles[k][:, ts(m, 128)], xt[:, k * S : k * S + 512],
                                start=(k == 0), stop=(k == ND - 1),
                            )
                            i2 = nc.tensor.matmul(
                                ps[:, 512:S], wtiles[k][:, ts(m, 128)], xt[:, k * S + 512 : k * S + S],
                                start=(k == 0), stop=(k == ND - 1),
                            )
                            i2.ins.ldweights = False
                        # evac + bias add (per-partition bias in transposed layout)
                        nc.vector.tensor_scalar_add(dst[m], ps[:, 0:S], bias_t[:, m : m + 1])
                    if m < NT:
                        t0, st = S_TILES[m]
                        ps = ps_big.tile([128, 1024], F32, tag="big", name="psbig")
                        for k in range(ND):
                            nc.tensor.matmul(
                                ps[:st, 0:512], xt[:, k * S + t0 : k * S + t0 + st], wvb[k][:, 0:512],
                                start=(k == 0), stop=(k == ND - 1),
                            )
                            i2 = nc.tensor.matmul(
                                ps[:st, 512:1024], xt[:, k * S + t0 : k * S + t0 + st], wvb[k][:, 512:1024],
                                start=(k == 0), stop=(k == ND - 1),
                            )
                            i2.ins.ldweights = False
                        v3 = vt[m].rearrange("p (h c) -> p h c", c=65)
                        nc.vector.tensor_copy(
                            out=v3[:st, :, 0:64],
                            in_=ps[:st].rearrange("p (h c) -> p h c", c=64),
                        )
                        nc.vector.memset(v3[:, :, 64:65], 1.0)
                    yield

            def emit_ctx(b, p, ptiles, ost, vt):
                # ctx^T = V^T P^T with V stationary (5 weight loads per head
                # instead of 25), then 5 PE transposes back to natural layout.
                ctxT = []
                for half in range(2):
                    h = 2 * p + half
                    psT = ps_big.tile([128, 1024], F32, tag="big", name="psbig")
                    for t, (t0, st) in enumerate(S_TILES):
                        nc.tensor.matmul(
                            psT[:65, 0:512], vt[t][:st, ds(65 * h, 65)],
                            ptiles[half][t][:st, 0:512],
                            start=(t == 0), stop=(t == NT - 1),
                        )
                        i2 = nc.tensor.matmul(
                            psT[:65, 512:S], vt[t][:st, ds(65 * h, 65)],
                            ptiles[half][t][:st, 512:S],
                            start=(t == 0), stop=(t == NT - 1),
                        )
                        i2.ins.ldweights = False
                    ct = ctxT_pool.tile([128, S], BF16, tag="ctxT", name="ctxT")
                    nc.vector.tensor_copy(out=ct[:65], in_=psT[:65, 0:S])
                    ctxT.append(ct)
                for half in range(2):
                    h = 2 * p + half
                    ct = ctxT[half]
                    # 66-col blocks keep bf16 PSUM accesses 4-byte aligned
                    psc = ps_ctx.tile([128, 512], BF16, tag="ctx", name="psctx")
                    for j, (j0, sj) in enumerate(S_TILES):
                        nc.tensor.transpose(
                            psc[:sj, ds(66 * j, 65)], ct[:65, j0 : j0 + sj], identity[:65, :65]
                        )
                    # batched reciprocal of the 5 denominator columns (64::66)
                    rc = rc_pool.tile([128, NT], F32, tag="rc", name="rctile")
                    nc.vector.reciprocal(
                        rc[:, 0:NT].rearrange("p (j c) -> p j c", c=1),
                        psc[:, 0 : 66 * NT].rearrange("p (j c) -> p j c", c=66)[:, :, 64:65],
                    )
                    for j, (j0, sj) in enumerate(S_TILES):
                        nc.vector.scalar_tensor_tensor(
                            out=ost[j][:sj, ds(64 * h, 64)],
                            in0=psc[:sj, ds(66 * j, 64)],
                            scalar=rc[:sj, j : j + 1],
                            in1=bvb[:sj, ds(64 * h, 64)],
                            op0=OP.mult,
                            op1=OP.add,
                        )

            def phase2(b, tl):
                """Attention per head pair + store. Yields 8 times."""
                qt, kt, vt = tl["qt"], tl["kt"], tl["vt"]
                ost = [o_pool.tile([128, D], F32, tag=f"o{j}", name=f"otile{j}") for j in range(NT)]
                prev = None
                for p in range(HPAIRS):
                    ptiles = ([], [])
                    for t, (t0, st) in enumerate(S_TILES):
                        for half in range(2):
                            h0 = half * 64
                            psS = ps_big.tile([128, 1024], F32, tag="big", name="psbig")
                            nc.tensor.matmul(
                                psS[:st, 0:512],
                                kt[p][h0 : h0 + 64, t0 : t0 + st],
                                qt[p][h0 : h0 + 64, 0:512],
                                start=True, stop=True, tile_position=(h0, 0),
                            )
                            i2 = nc.tensor.matmul(
                                psS[:st, 512:S],
                                kt[p][h0 : h0 + 64, t0 : t0 + st],
                                qt[p][h0 : h0 + 64, 512:S],
                                start=True, stop=True, tile_position=(h0, 0),
                            )
                            i2.ins.ldweights = False
                            pT = pT_pool.tile([128, S], BF16, tag="pT", name="pTtile")
                            nc.scalar.activation(pT[:st], psS[:st, 0:S], AF.Exp, scale=0.125)
                            ptiles[half].append(pT)
                    if prev is not None:
                        emit_ctx(b, prev[0], prev[1], ost, vt)
                    prev = (p, ptiles)
                    yield
                emit_ctx(b, prev[0], prev[1], ost, vt)
                for j, (j0, sj) in enumerate(S_TILES):
                    nc.sync.dma_start(out=out[b, j0 : j0 + sj, :], in_=ost[j][:sj])

            # --- software-pipelined batch loop ---
            prev_attn = None
            for b in range(BPC):
                tl = make_tiles()
                g01 = phase01(b, tl)
                if prev_attn is None:
                    for _ in g01:
                        pass
                else:
                    gens = [prev_attn, g01]
                    while gens:
                        for g in list(gens):
                            try:
                                next(g)
                            except StopIteration:
                                gens.remove(g)
                prev_attn = phase2(b, tl)
            for _ in prev_attn:
                pass

    return nc


_NC = None


def make_in_maps(hidden_states, Wq, bq, Wk, bk, Wv, bv):
    bf = ml_dtypes.bfloat16
    hs = np.ascontiguousarray(np.asarray(hidden_states, np.float32).astype(bf))
    args = {
        "Wq": np.ascontiguousarray(np.asarray(Wq, np.float32).astype(bf)),
        "bq": np.ascontiguousarray(np.asarray(bq, np.float32)),
        "Wk": np.ascontiguousarray(np.asarray(Wk, np.float32).astype(bf)),
        "bk": np.ascontiguousarray(np.asarray(bk, np.float32)),
        "Wv": np.ascontiguousarray(np.asarray(Wv, np.float32).astype(bf)),
        "bv": np.ascontiguousarray(np.asarray(bv, np.float32)),
    }
    return [
        {"hidden": hs[i * BPC : (i + 1) * BPC], **args} for i in range(N_CORES)
    ]


def kernel(hidden_states, Wq, bq, Wk, bk, Wv, bv):
    global _NC
    if _NC is None:
        _NC = build_nc()
    in_maps = make_in_maps(hidden_states, Wq, bq, Wk, bk, Wv, bv)
    res = run_bass_kernel_spmd(_NC, in_maps, list(range(N_CORES)))
    return np.concatenate([res.results[i]["out"] for i in range(N_CORES)], axis=0)


# revision 15
# speedup vs baseline: 1.0111x; 1.0111x over previous
"""ViT self-attention (B=32, S=577, D=1024, H=16, Dh=64) on 8 TRN2 NeuronCores.

Sharding: data-parallel over batch — each core gets 4 batch elements, no
collectives. All matmuls run in bf16 (1 PE cycle/row vs 4 for fp32); inputs
are cast to bf16 on the host so DMA traffic halves and no on-device convert
is needed. Weights are loaded to SBUF once per core (not per batch).

Per core, per batch:
  phase 0: PE-transpose X [577,1024] -> X^T stored as one [128, 8*577] bf16
           tile (one DVE evac per 128-token tile)
  phase 1: Q^T = Wq^T X^T, K^T = Wk^T X^T (bias folded into DVE evac),
           V natural = X Wv with a ones column per head ([V_h | 1] -> softmax
           denominator falls out of the ctx matmul)
  phase 2: per head pair (row-packed K=64 matmuls at tile_position (0,0)/
           (64,0)): S^T tile = matmul(lhsT=K^T, rhs=Q^T); P^T = exp(S^T/8) on
           ACT (bf16); ctx natural = matmul(lhsT=P^T, rhs=[V_h|1]) in PSUM
           with denominator in col 64; DVE: batched recip + fused
           (ctx*recip + bv) evac; DMA out per 128-token tile.

The batch loop is software-pipelined: batch b's attention (ACT-heavy exp)
is interleaved with batch b+1's QKV projections (PE-heavy) so the scalar
engine's exp time hides under tensor-engine work.
"""

import numpy as np
import ml_dtypes

import concourse.bass as bass
import concourse.mybir as mybir
import concourse.tile as tile
from concourse.bass import ds, ts
from concourse.bass_utils import run_bass_kernel_spmd
from concourse.masks import make_identity

F32 = mybir.dt.float32
BF16 = mybir.dt.bfloat16

# ---------------------------------------------------------------------------
# Wait-legalization patch: this walrus build accepts at most ONE ge-mode sync
# wait per instruction (eq-mode counts as two). Tile's sem assignment attaches
# multi-waits directly to instructions, so hoist extras onto standalone
# EventSemaphore carriers (same engine queue, immediately preceding — identical
# semantics, queue is in-order).
# ---------------------------------------------------------------------------
_ctr = [0]


def _split_waits(insts):
    out = []
    for inst in insts:
        si = inst.sync_info
        if si is not None and si.on_wait:
            waits = list(si.on_wait)
            if len(waits) == 1 and waits[0].wait_mode != "sem-eq-imm":
                move = []
            else:
                move = waits
            for w in move:
                _ctr[0] += 1
                ev = mybir.InstEventSemaphore(
                    name=f"wsplit_{_ctr[0]}", opcode="EventSemaphore",
                    engine=inst.engine, debug=inst.debug, ins=[], outs=[],
                    sync_info=mybir.SyncInfo(on_wait=[w], on_update=[]),
                )
                out.append(ev)
            if move:
                inst.sync_info = mybir.SyncInfo(on_wait=[], on_update=list(si.on_update))
        out.append(inst)
    return out


def _install_waitfix():
    if getattr(tile.TileContext, "_waitfix_installed", False):
        return
    from concourse.vector_clock import ScopedClock

    orig_lower = tile.TileContext._lower_ordered_insts

    def patched_lower(self, ordered):
        for name in list(ordered.keys()):
            ordered[name] = _split_waits(ordered[name])
        return orig_lower(self, ordered)

    def patched_dab(self, tick_clock, wait_clock):
        nc = self.nc
        probe = nc.sync.nop(nofuse=True)
        wait_clock.add_sem_waits(probe.ins, ScopedClock({None: tick_clock.global_clock}))
        si = probe.ins.sync_info
        waits = list(si.on_wait) if si is not None else []
        probe.ins.sync_info = mybir.SyncInfo(
            on_wait=[], on_update=list(si.on_update) if si else []
        )
        for w in waits:
            _ctr[0] += 1
            ev = mybir.InstEventSemaphore(
                name=f"wsplit_dab_{_ctr[0]}", opcode="EventSemaphore",
                engine=mybir.EngineType.SP, debug=probe.ins.debug, ins=[], outs=[],
                sync_info=mybir.SyncInfo(on_wait=[w], on_update=[]),
            )
            nc.sync.add_instruction(ev)
        nc.sync.drain()
        nc.all_engine_barrier()
        assert self.sems is not None
        popped = nc._tile_sem_poison_stack.pop()
        assert popped is self._sem_poison
        nc.clear_and_free_semaphores(list(self.sems.allocated().values()))
        nc.all_engine_barrier()

    tile.TileContext._lower_ordered_insts = patched_lower
    tile.TileContext._drain_and_barrier = patched_dab
    tile.TileContext._waitfix_installed = True


_install_waitfix()

N_CORES = 8
B, S, D = 32, 577, 1024
H, Dh = 16, 64
BPC = B // N_CORES  # batches per core
S_TILES = [(t * 128, min(128, S - t * 128)) for t in range((S + 127) // 128)]  # 5 tiles
ND = D // 128  # 8 din/dout tiles
HPAIRS = H // 2
NT = len(S_TILES)

AF = mybir.ActivationFunctionType
OP = mybir.AluOpType


def build_nc():
    nc = bass.Bass()
    hidden = nc.declare_dram_parameter("hidden", [BPC, S, D], BF16, isOutput=False)
    wq = nc.declare_dram_parameter("Wq", [D, D], BF16, isOutput=False)
    bq = nc.declare_dram_parameter("bq", [D], F32, isOutput=False)
    wk = nc.declare_dram_parameter("Wk", [D, D], BF16, isOutput=False)
    bk = nc.declare_dram_parameter("bk", [D], F32, isOutput=False)
    wv = nc.declare_dram_parameter("Wv", [D, D], BF16, isOutput=False)
    bv = nc.declare_dram_parameter("bv", [D], F32, isOutput=False)
    out = nc.declare_dram_parameter("out", [BPC, S, D], F32, isOutput=True)

    with tile.TileContext(nc) as tc:
        with (
            tc.tile_pool(name="singles", bufs=1) as singles,
            tc.tile_pool(name="xnat", bufs=3) as xnat_pool,
            tc.tile_pool(name="xt", bufs=1) as xt_pool,
            tc.tile_pool(name="qt", bufs=2) as qt_pool,
            tc.tile_pool(name="kt", bufs=2) as kt_pool,
            tc.tile_pool(name="v", bufs=2) as v_pool,
            tc.tile_pool(name="pT", bufs=22) as pT_pool,
            tc.tile_pool(name="ctxT", bufs=4) as ctxT_pool,
            tc.tile_pool(name="ostage", bufs=1) as o_pool,
            tc.tile_pool(name="rc", bufs=8) as rc_pool,
            tc.tile_pool(name="psbig", bufs=3, space="PSUM") as ps_big,
            tc.tile_pool(name="psctx", bufs=2, space="PSUM") as ps_ctx,
        ):
            # --- constants ---
            identity = singles.tile([128, 128], BF16)
            make_identity(nc, identity)
            # per-dout-tile bias columns: bqt[:, m] = bq[128m : 128(m+1)]
            bqt = singles.tile([128, ND], F32)
            bkt = singles.tile([128, ND], F32)
            nc.gpsimd.dma_start(out=bqt, in_=bq[:].rearrange("(m p) -> p m", p=128))
            nc.gpsimd.dma_start(out=bkt, in_=bk[:].rearrange("(m p) -> p m", p=128))
            # bv broadcast to all 128 partitions
            bvb = singles.tile([128, D], F32)
            bv_ap = bv[:]
            nc.gpsimd.dma_start(
                out=bvb,
                in_=bass.AP(tensor=bv_ap.tensor, offset=bv_ap.offset, ap=[[0, 128]] + bv_ap.ap),
            )
            # weights: loaded once, reused for all batches
            wqb = [singles.tile([128, D], BF16, name=f"wq{k}") for k in range(ND)]
            wkb = [singles.tile([128, D], BF16, name=f"wk{k}") for k in range(ND)]
            wvb = [singles.tile([128, D], BF16, name=f"wv{k}") for k in range(ND)]
            for wdram, wtiles in ((wq, wqb), (wk, wkb), (wv, wvb)):
                for k in range(ND):
                    nc.gpsimd.dma_start(out=wtiles[k], in_=wdram[ts(k, 128), :])

            def make_tiles():
                return {
                    "xt": xt_pool.tile([128, ND * S], BF16, tag="xt", name="xt"),
                    "qt": [qt_pool.tile([128, S], BF16, tag=f"qt{m}", name=f"qtile{m}") for m in range(ND)],
                    "kt": [kt_pool.tile([128, S], BF16, tag=f"kt{m}", name=f"ktile{m}") for m in range(ND)],
                    "vt": [v_pool.tile([128, H * 65], BF16, tag=f"v{t}", name=f"vtile{t}") for t in range(NT)],
                }

            def phase01(b, tl):
                """X load + transpose, then QKV projections. Yields 9 times."""
                xt, qt, kt, vt = tl["xt"], tl["qt"], tl["kt"], tl["vt"]
                x3 = xt.rearrange("p (j c) -> p j c", c=S)
                for t, (t0, st) in enumerate(S_TILES):
                    xn = xnat_pool.tile([128, D], BF16, tag="xn")
                    nc.sync.dma_start(out=xn[:st], in_=hidden[b, t0 : t0 + st, :])
                    pst = ps_big.tile([128, 1024], BF16, tag="big", name="psbig")
                    for j in range(ND):
                        nc.tensor.transpose(
                            pst[:, j * 128 : j * 128 + st], xn[:st, ts(j, 128)], identity[:st, :st]
                        )
                    nc.vector.tensor_copy(
                        out=x3[:, :, t0 : t0 + st],
                        in_=pst.rearrange("p (j c) -> p j c", c=128)[:, :, 0:st],
                    )
                yield
                for m in range(ND):
                    for wtiles, dst, bias_t in ((wqb, qt, bqt), (wkb, kt, bkt)):
                        ps = ps_big.tile([128, 1024], F32, tag="big", name="psbig")
                        for k in range(ND):
                            nc.tensor.matmul(
                                ps[:, 0:512], wtiles[k][:, ts(m, 128)], xt[:, k * S : k * S + 512],
                                start=(k == 0), stop=(k == ND - 1),
                            )
                            nc.tensor.matmul(
                                ps[:, 512:S], wtiles[k][:, ts(m, 128)], xt[:, k * S + 512 : k * S + S],
                                start=(k == 0), stop=(k == ND - 1),
                            )
                        # evac + bias add (per-partition bias in transposed layout)
                        nc.vector.tensor_scalar_add(dst[m], ps[:, 0:S], bias_t[:, m : m + 1])
                    if m < NT:
                        t0, st = S_TILES[m]
                        ps = ps_big.tile([128, 1024], F32, tag="big", name="psbig")
                        for k in range(ND):
                            nc.tensor.matmul(
                                ps[:st, 0:512], xt[:, k * S + t0 : k * S + t0 + st], wvb[k][:, 0:512],
                                start=(k == 0), stop=(k == ND - 1),
                            )
                            nc.tensor.matmul(
                                ps[:st, 512:1024], xt[:, k * S + t0 : k * S + t0 + st], wvb[k][:, 512:1024],
                                start=(k == 0), stop=(k == ND - 1),
                            )
                        v3 = vt[m].rearrange("p (h c) -> p h c", c=65)
                        nc.vector.tensor_copy(
                            out=v3[:st, :, 0:64],
                            in_=ps[:st].rearrange("p (h c) -> p h c", c=64),
                        )
                        nc.vector.memset(v3[:, :, 64:65], 1.0)
                    yield

            def emit_ctxT_mm(b, p, ptiles, vt):
                """ctx^T = V^T P^T with V stationary (5 weight loads per head
                instead of 25) + evac to SBUF bf16. Returns the two ct tiles."""
                cts = []
                for half in range(2):
                    h = 2 * p + half
                    psT = ps_big.tile([128, 1024], F32, tag="big", name="psbig")
                    for t, (t0, st) in enumerate(S_TILES):
                        nc.tensor.matmul(
                            psT[:65, 0:512], vt[t][:st, ds(65 * h, 65)],
                            ptiles[half][t][:st, 0:512],
                            start=(t == 0), stop=(t == NT - 1),
                        )
                        nc.tensor.matmul(
                            psT[:65, 512:S], vt[t][:st, ds(65 * h, 65)],
                            ptiles[half][t][:st, 512:S],
                            start=(t == 0), stop=(t == NT - 1),
                        )
                    ct = ctxT_pool.tile([128, S], BF16, tag="ctxT", name="ctxT")
                    nc.vector.tensor_copy(out=ct[:65], in_=psT[:65, 0:S])
                    cts.append(ct)
                return cts

            def emit_ctx_fin(b, p, cts, ost):
                """Back-transpose ctx^T to natural layout, normalize, + bias."""
                for half in range(2):
                    h = 2 * p + half
                    ct = cts[half]
                    # 66-col blocks keep bf16 PSUM accesses 4-byte aligned
                    psc = ps_ctx.tile([128, 512], BF16, tag="ctx", name="psctx")
                    for j, (j0, sj) in enumerate(S_TILES):
                        nc.tensor.transpose(
                            psc[:sj, ds(66 * j, 65)], ct[:65, j0 : j0 + sj], identity[:65, :65]
                        )
                    # batched reciprocal of the 5 denominator columns (64::66)
                    rc = rc_pool.tile([128, NT], F32, tag="rc", name="rctile")
                    nc.vector.reciprocal(
                        rc[:, 0:NT].rearrange("p (j c) -> p j c", c=1),
                        psc[:, 0 : 66 * NT].rearrange("p (j c) -> p j c", c=66)[:, :, 64:65],
                    )
                    for j, (j0, sj) in enumerate(S_TILES):
                        nc.vector.scalar_tensor_tensor(
                            out=ost[j][:sj, ds(64 * h, 64)],
                            in0=psc[:sj, ds(66 * j, 64)],
                            scalar=rc[:sj, j : j + 1],
                            in1=bvb[:sj, ds(64 * h, 64)],
                            op0=OP.mult,
                            op1=OP.add,
                        )

            def phase2(b, tl):
                """Attention per head pair + store. Yields 8 times. Two-deep
                pipeline: chunk p emits scores(p), ctx^T matmuls(p-1), and the
                back-transpose + normalize of p-2, so the DVE evac of ctx^T
                never stalls the PE."""
                qt, kt, vt = tl["qt"], tl["kt"], tl["vt"]
                ost = [o_pool.tile([128, D], F32, tag=f"o{j}", name=f"otile{j}") for j in range(NT)]
                prev = None  # (p, ptiles)
                done = None  # (p, cts)
                for p in range(HPAIRS):
                    ptiles = ([], [])
                    for t, (t0, st) in enumerate(S_TILES):
                        for half in range(2):
                            h0 = half * 64
                            psS = ps_big.tile([128, 1024], F32, tag="big", name="psbig")
                            nc.tensor.matmul(
                                psS[:st, 0:512],
                                kt[p][h0 : h0 + 64, t0 : t0 + st],
                                qt[p][h0 : h0 + 64, 0:512],
                                start=True, stop=True, tile_position=(h0, 0),
                            )
                            nc.tensor.matmul(
                                psS[:st, 512:S],
                                kt[p][h0 : h0 + 64, t0 : t0 + st],
                                qt[p][h0 : h0 + 64, 512:S],
                                start=True, stop=True, tile_position=(h0, 0),
                            )
                            pT = pT_pool.tile([128, S], BF16, tag="pT", name="pTtile")
                            nc.scalar.activation(pT[:st], psS[:st, 0:S], AF.Exp, scale=0.125)
                            ptiles[half].append(pT)
                    if prev is not None:
                        cts = emit_ctxT_mm(b, prev[0], prev[1], vt)
                        if done is not None:
                            emit_ctx_fin(b, done[0], done[1], ost)
                        done = (prev[0], cts)
                    prev = (p, ptiles)
                    yield
                cts = emit_ctxT_mm(b, prev[0], prev[1], vt)
                emit_ctx_fin(b, done[0], done[1], ost)
                emit_ctx_fin(b, prev[0], cts, ost)
                for j, (j0, sj) in enumerate(S_TILES):
                    nc.sync.dma_start(out=out[b, j0 : j0 + sj, :], in_=ost[j][:sj])

            # --- software-pipelined batch loop ---
            prev_attn = None
            for b in range(BPC):
                tl = make_tiles()
                g01 = phase01(b, tl)
                if prev_attn is None:
                    for _ in g01:
                        pass
                else:
                    gens = [prev_attn, g01]
                    while gens:
                        for g in list(gens):
                            try:
                                next(g)
                            except StopIteration:
                                gens.remove(g)
                prev_attn = phase2(b, tl)
            for _ in prev_attn:
                pass

    return nc


_NC = None


def make_in_maps(hidden_states, Wq, bq, Wk, bk, Wv, bv):
    bf = ml_dtypes.bfloat16
    hs = np.ascontiguousarray(np.asarray(hidden_states, np.float32).astype(bf))
    args = {
        "Wq": np.ascontiguousarray(np.asarray(Wq, np.float32).astype(bf)),
        "bq": np.ascontiguousarray(np.asarray(bq, np.float32)),
        "Wk": np.ascontiguousarray(np.asarray(Wk, np.float32).astype(bf)),
        "bk": np.ascontiguousarray(np.asarray(bk, np.float32)),
        "Wv": np.ascontiguousarray(np.asarray(Wv, np.float32).astype(bf)),
        "bv": np.ascontiguousarray(np.asarray(bv, np.float32)),
    }
    return [
        {"hidden": hs[i * BPC : (i + 1) * BPC], **args} for i in range(N_CORES)
    ]


def kernel(hidden_states, Wq, bq, Wk, bk, Wv, bv):
    global _NC
    if _NC is None:
        _NC = build_nc()
    in_maps = make_in_maps(hidden_states, Wq, bq, Wk, bk, Wv, bv)
    res = run_bass_kernel_spmd(_NC, in_maps, list(range(N_CORES)))
    return np.concatenate([res.results[i]["out"] for i in range(N_CORES)], axis=0)


# revision 19
# speedup vs baseline: 1.0437x; 1.0323x over previous
"""ViT self-attention (B=32, S=577, D=1024, H=16, Dh=64) on 8 TRN2 NeuronCores.

Sharding: data-parallel over batch — each core gets 4 batch elements, no
collectives. All matmuls run in bf16 (1 PE cycle/row vs 4 for fp32); inputs
are cast to bf16 on the host so DMA traffic halves and no on-device convert
is needed. Weights are loaded to SBUF once per core (not per batch).

Per core, per batch:
  phase 0: PE-transpose X [577,1024] -> X^T stored as one [128, 8*577] bf16
           tile (one DVE evac per 128-token tile)
  phase 1: Q^T = Wq^T X^T, K^T = Wk^T X^T (bias folded into DVE evac),
           V natural = X Wv with a ones column per head ([V_h | 1] -> softmax
           denominator falls out of the ctx matmul)
  phase 2: per head pair (row-packed K=64 matmuls at tile_position (0,0)/
           (64,0)): S^T tile = matmul(lhsT=K^T, rhs=Q^T); P^T = exp(S^T/8) on
           ACT (bf16); ctx natural = matmul(lhsT=P^T, rhs=[V_h|1]) in PSUM
           with denominator in col 64; DVE: batched recip + fused
           (ctx*recip + bv) evac; DMA out per 128-token tile.

The batch loop is software-pipelined: batch b's attention (ACT-heavy exp)
is interleaved with batch b+1's QKV projections (PE-heavy) so the scalar
engine's exp time hides under tensor-engine work.
"""

import numpy as np
import ml_dtypes

import concourse.bass as bass
import concourse.mybir as mybir
import concourse.tile as tile
from concourse.bass import ds, ts
from concourse.bass_utils import run_bass_kernel_spmd
from concourse.masks import make_identity

F32 = mybir.dt.float32
BF16 = mybir.dt.bfloat16

# ---------------------------------------------------------------------------
# Wait-legalization patch: this walrus build accepts at most ONE ge-mode sync
# wait per instruction (eq-mode counts as two). Tile's sem assignment attaches
# multi-waits directly to instructions, so hoist extras onto standalone
# EventSemaphore carriers (same engine queue, immediately preceding — identical
# semantics, queue is in-order).
# ---------------------------------------------------------------------------
_ctr = [0]


def _split_waits(insts):
    out = []
    for inst in insts:
        si = inst.sync_info
        if si is not None and si.on_wait:
            waits = list(si.on_wait)
            if len(waits) == 1 and waits[0].wait_mode != "sem-eq-imm":
                move = []
            else:
                move = waits
            for w in move:
                _ctr[0] += 1
                ev = mybir.InstEventSemaphore(
                    name=f"wsplit_{_ctr[0]}", opcode="EventSemaphore",
                    engine=inst.engine, debug=inst.debug, ins=[], outs=[],
                    sync_info=mybir.SyncInfo(on_wait=[w], on_update=[]),
                )
                out.append(ev)
            if move:
                inst.sync_info = mybir.SyncInfo(on_wait=[], on_update=list(si.on_update))
        out.append(inst)
    return out


def _install_waitfix():
    if getattr(tile.TileContext, "_waitfix_installed", False):
        return
    from concourse.vector_clock import ScopedClock

    orig_lower = tile.TileContext._lower_ordered_insts

    def patched_lower(self, ordered):
        for name in list(ordered.keys()):
            ordered[name] = _split_waits(ordered[name])
        return orig_lower(self, ordered)

    def patched_dab(self, tick_clock, wait_clock):
        nc = self.nc
        probe = nc.sync.nop(nofuse=True)
        wait_clock.add_sem_waits(probe.ins, ScopedClock({None: tick_clock.global_clock}))
        si = probe.ins.sync_info
        waits = list(si.on_wait) if si is not None else []
        probe.ins.sync_info = mybir.SyncInfo(
            on_wait=[], on_update=list(si.on_update) if si else []
        )
        for w in waits:
            _ctr[0] += 1
            ev = mybir.InstEventSemaphore(
                name=f"wsplit_dab_{_ctr[0]}", opcode="EventSemaphore",
                engine=mybir.EngineType.SP, debug=probe.ins.debug, ins=[], outs=[],
                sync_info=mybir.SyncInfo(on_wait=[w], on_update=[]),
            )
            nc.sync.add_instruction(ev)
        nc.sync.drain()
        nc.all_engine_barrier()
        assert self.sems is not None
        popped = nc._tile_sem_poison_stack.pop()
        assert popped is self._sem_poison
        nc.clear_and_free_semaphores(list(self.sems.allocated().values()))
        nc.all_engine_barrier()

    tile.TileContext._lower_ordered_insts = patched_lower
    tile.TileContext._drain_and_barrier = patched_dab
    tile.TileContext._waitfix_installed = True


_install_waitfix()

N_CORES = 8
B, S, D = 32, 577, 1024
H, Dh = 16, 64
BPC = B // N_CORES  # batches per core
S_TILES = [(t * 128, min(128, S - t * 128)) for t in range((S + 127) // 128)]  # 5 tiles
ND = D // 128  # 8 din/dout tiles
HPAIRS = H // 2
NT = len(S_TILES)

AF = mybir.ActivationFunctionType
OP = mybir.AluOpType


def build_nc():
    nc = bass.Bass()
    hidden = nc.declare_dram_parameter("hidden", [BPC, S, D], BF16, isOutput=False)
    wq = nc.declare_dram_parameter("Wq", [D, D], BF16, isOutput=False)
    bq = nc.declare_dram_parameter("bq", [D], F32, isOutput=False)
    wk = nc.declare_dram_parameter("Wk", [D, D], BF16, isOutput=False)
    bk = nc.declare_dram_parameter("bk", [D], F32, isOutput=False)
    wv = nc.declare_dram_parameter("Wv", [D, D], BF16, isOutput=False)
    bv = nc.declare_dram_parameter("bv", [D], F32, isOutput=False)
    out = nc.declare_dram_parameter("out", [BPC, S, D], F32, isOutput=True)

    with tile.TileContext(nc) as tc:
        with (
            tc.tile_pool(name="singles", bufs=1) as singles,
            tc.tile_pool(name="xnat", bufs=3) as xnat_pool,
            tc.tile_pool(name="xt", bufs=1) as xt_pool,
            tc.tile_pool(name="qt", bufs=1) as qt_pool,
            tc.tile_pool(name="kt", bufs=1) as kt_pool,
            tc.tile_pool(name="v", bufs=1) as v_pool,
            tc.tile_pool(name="pT", bufs=22) as pT_pool,
            tc.tile_pool(name="ostage", bufs=1) as o_pool,
            tc.tile_pool(name="rc", bufs=8) as rc_pool,
            tc.tile_pool(name="psbig", bufs=3, space="PSUM") as ps_big,
            tc.tile_pool(name="psctx", bufs=2, space="PSUM") as ps_ctx,
        ):
            # --- constants ---
            identity = singles.tile([128, 128], BF16)
            make_identity(nc, identity)
            # per-dout-tile bias columns: bqt[:, m] = bq[128m : 128(m+1)]
            bqt = singles.tile([128, ND], F32)
            bkt = singles.tile([128, ND], F32)
            nc.gpsimd.dma_start(out=bqt, in_=bq[:].rearrange("(m p) -> p m", p=128))
            nc.gpsimd.dma_start(out=bkt, in_=bk[:].rearrange("(m p) -> p m", p=128))
            # bv broadcast to all 128 partitions
            bvb = singles.tile([128, D], F32)
            bv_ap = bv[:]
            nc.gpsimd.dma_start(
                out=bvb,
                in_=bass.AP(tensor=bv_ap.tensor, offset=bv_ap.offset, ap=[[0, 128]] + bv_ap.ap),
            )
            # weights: loaded once, reused for all batches
            wqb = [singles.tile([128, D], BF16, name=f"wq{k}") for k in range(ND)]
            wkb = [singles.tile([128, D], BF16, name=f"wk{k}") for k in range(ND)]
            wvb = [singles.tile([128, D], BF16, name=f"wv{k}") for k in range(ND)]
            for wdram, wtiles in ((wq, wqb), (wk, wkb), (wv, wvb)):
                for k in range(ND):
                    nc.gpsimd.dma_start(out=wtiles[k], in_=wdram[ts(k, 128), :])

            def emit_ctx(b, p, ptiles, ost, vt):
                for half in range(2):
                    h = 2 * p + half
                    psc = ps_ctx.tile([128, 512], F32, tag="ctx", name="psctx")
                    for j, (j0, sj) in enumerate(S_TILES):
                        for t, (t0, st) in enumerate(S_TILES):
                            nc.tensor.matmul(
                                psc[:sj, ds(65 * j, 65)],
                                ptiles[half][t][:st, j0 : j0 + sj],
                                vt[t][:st, ds(65 * h, 65)],
                                start=(t == 0), stop=(t == NT - 1),
                            )
                    # batched reciprocal of the 5 denominator columns (64::65)
                    rc = rc_pool.tile([128, NT], F32, tag="rc", name="rctile")
                    nc.vector.reciprocal(
                        rc[:, 0:NT].rearrange("p (j c) -> p j c", c=1),
                        psc[:, 0 : 65 * NT].rearrange("p (j c) -> p j c", c=65)[:, :, 64:65],
                    )
                    for j, (j0, sj) in enumerate(S_TILES):
                        nc.vector.scalar_tensor_tensor(
                            out=ost[j][:sj, ds(64 * h, 64)],
                            in0=psc[:sj, ds(65 * j, 64)],
                            scalar=rc[:sj, j : j + 1],
                            in1=bvb[:sj, ds(64 * h, 64)],
                            op0=OP.mult,
                            op1=OP.add,
                        )

            def emit_transposes(b, xt):
                x3 = xt.rearrange("p (j c) -> p j c", c=S)
                for t, (t0, st) in enumerate(S_TILES):
                    xn = xnat_pool.tile([128, D], BF16, tag="xn")
                    nc.sync.dma_start(out=xn[:st], in_=hidden[b, t0 : t0 + st, :])
                    pst = ps_big.tile([128, 1024], BF16, tag="big", name="psbig")
                    for j in range(ND):
                        nc.tensor.transpose(
                            pst[:, j * 128 : j * 128 + st], xn[:st, ts(j, 128)], identity[:st, :st]
                        )
                    nc.vector.tensor_copy(
                        out=x3[:, :, t0 : t0 + st],
                        in_=pst.rearrange("p (j c) -> p j c", c=128)[:, :, 0:st],
                    )

            def emit_V(b, xt, vt, tlist):
                for t in tlist:
                    t0, st = S_TILES[t]
                    ps = ps_big.tile([128, 1024], F32, tag="big", name="psbig")
                    for k in range(ND):
                        nc.tensor.matmul(
                            ps[:st, 0:512], xt[:, k * S + t0 : k * S + t0 + st], wvb[k][:, 0:512],
                            start=(k == 0), stop=(k == ND - 1),
                        )
                        nc.tensor.matmul(
                            ps[:st, 512:1024], xt[:, k * S + t0 : k * S + t0 + st], wvb[k][:, 512:1024],
                            start=(k == 0), stop=(k == ND - 1),
                        )
                    v3 = vt[t].rearrange("p (h c) -> p h c", c=65)
                    nc.vector.tensor_copy(
                        out=v3[:st, :, 0:64],
                        in_=ps[:st].rearrange("p (h c) -> p h c", c=64),
                    )
                    nc.vector.memset(v3[:, :, 64:65], 1.0)

            def emit_QK(b, xt, qt, kt, m):
                for wtiles, dst, bias_t in ((wqb, qt, bqt), (wkb, kt, bkt)):
                    ps = ps_big.tile([128, 1024], F32, tag="big", name="psbig")
                    for k in range(ND):
                        nc.tensor.matmul(
                            ps[:, 0:512], wtiles[k][:, ts(m, 128)], xt[:, k * S : k * S + 512],
                            start=(k == 0), stop=(k == ND - 1),
                        )
                        nc.tensor.matmul(
                            ps[:, 512:S], wtiles[k][:, ts(m, 128)], xt[:, k * S + 512 : k * S + S],
                            start=(k == 0), stop=(k == ND - 1),
                        )
                    # evac + bias add (per-partition bias in transposed layout)
                    nc.vector.tensor_scalar_add(dst[m], ps[:, 0:S], bias_t[:, m : m + 1])

            def emit_scores_exp(b, qt, kt, p):
                ptiles = ([], [])
                for t, (t0, st) in enumerate(S_TILES):
                    for half in range(2):
                        h0 = half * 64
                        psS = ps_big.tile([128, 1024], F32, tag="big", name="psbig")
                        nc.tensor.matmul(
                            psS[:st, 0:512],
                            kt[p][h0 : h0 + 64, t0 : t0 + st],
                            qt[p][h0 : h0 + 64, 0:512],
                            start=True, stop=True, tile_position=(h0, 0),
                        )
                        nc.tensor.matmul(
                            psS[:st, 512:S],
                            kt[p][h0 : h0 + 64, t0 : t0 + st],
                            qt[p][h0 : h0 + 64, 512:S],
                            start=True, stop=True, tile_position=(h0, 0),
                        )
                        pT = pT_pool.tile([128, S], BF16, tag="pT", name="pTtile")
                        nc.scalar.activation(pT[:st], psS[:st, 0:S], AF.Exp, scale=0.125)
                        ptiles[half].append(pT)
                return ptiles

            def stream(b):
                """One batch, self-overlapped: attention pair p interleaves
                with the Q/K projection for pair p+1 so ACT exp time hides
                under PE work within the batch."""
                xt = xt_pool.tile([128, ND * S], BF16, tag="xt", name="xt")
                qt = [qt_pool.tile([128, S], BF16, tag=f"qt{m}", name=f"qtile{m}") for m in range(ND)]
                kt = [kt_pool.tile([128, S], BF16, tag=f"kt{m}", name=f"ktile{m}") for m in range(ND)]
                vt = [v_pool.tile([128, H * 65], BF16, tag=f"v{t}", name=f"vtile{t}") for t in range(NT)]
                ost = [o_pool.tile([128, D], F32, tag=f"o{j}", name=f"otile{j}") for j in range(NT)]
                emit_transposes(b, xt)
                yield
                emit_V(b, xt, vt, [0, 1])
                yield
                emit_V(b, xt, vt, [2, 3])
                yield
                emit_V(b, xt, vt, [4])
                emit_QK(b, xt, qt, kt, 0)
                yield
                prev = None
                for p in range(HPAIRS):
                    ptiles = emit_scores_exp(b, qt, kt, p)
                    if p + 1 < HPAIRS:
                        emit_QK(b, xt, qt, kt, p + 1)
                    if prev is not None:
                        emit_ctx(b, prev[0], prev[1], ost, vt)
                    prev = (p, ptiles)
                    yield
                emit_ctx(b, prev[0], prev[1], ost, vt)
                for j, (j0, sj) in enumerate(S_TILES):
                    nc.sync.dma_start(out=out[b, j0 : j0 + sj, :], in_=ost[j][:sj])

            for b in range(BPC):
                for _ in stream(b):
                    pass

    return nc


_NC = None


def make_in_maps(hidden_states, Wq, bq, Wk, bk, Wv, bv):
    bf = ml_dtypes.bfloat16
    hs = np.ascontiguousarray(np.asarray(hidden_states, np.float32).astype(bf))
    args = {
        "Wq": np.ascontiguousarray(np.asarray(Wq, np.float32).astype(bf)),
        "bq": np.ascontiguousarray(np.asarray(bq, np.float32)),
        "Wk": np.ascontiguousarray(np.asarray(Wk, np.float32).astype(bf)),
        "bk": np.ascontiguousarray(np.asarray(bk, np.float32)),
        "Wv": np.ascontiguousarray(np.asarray(Wv, np.float32).astype(bf)),
        "bv": np.ascontiguousarray(np.asarray(bv, np.float32)),
    }
    return [
        {"hidden": hs[i * BPC : (i + 1) * BPC], **args} for i in range(N_CORES)
    ]


def kernel(hidden_states, Wq, bq, Wk, bk, Wv, bv):
    global _NC
    if _NC is None:
        _NC = build_nc()
    in_maps = make_in_maps(hidden_states, Wq, bq, Wk, bk, Wv, bv)
    res = run_bass_kernel_spmd(_NC, in_maps, list(range(N_CORES)))
    return np.concatenate([res.results[i]["out"] for i in range(N_CORES)], axis=0)
